# revision 19
# baseline (speedup 1.0000x reference)
"""Trainium2 Bass kernel for nn_ModelNew_3556232921872 (dense_cnn).

Pipeline per sample:
  x_conv = conv3x3(x, W) + b
  acc    = depthwise3x3(x_conv, diag(W)) + b
  group stats over channels per pixel -> norm = (acc - mean_c) * rsqrt(var+eps)
  norm = norm * gamma + beta
  fused = tanh(norm) * clip(norm/6 + 0.5, 0, 1)
  out   = logsumexp(x_conv + fused, channels)          # [1, H, W]

Sharding: data-parallel over batch, B=16 -> 2 samples per NeuronCore x 8.

v2 design (vs v1 baseline):
 - M=128 row-interleaved packing: each 4-row block is split into group
   g=0 (even rows j, j+2) and g=1 (odd rows j+1, j+3), stacked in the
   output-partition dim.  Conv: 8 passes (4x K=128 dual-dx-copy + 4x
   K=64 dx=2), depthwise: 9 fully-M-packed passes over a redundant
   per-group row layout xcvL[(c,g), s] = x_conv[c, j+g+s-1], stats: 2
   passes, LSE: 1 pass -> 20 matmuls/block vs 34 in v1.
 - All elementwise work on 128 partitions (two row-groups at once).
 - Single ACT table (ln+exp): rsqrt = exp(-0.5 ln v + ln gamma) with
   gamma folded into the bias; tanh via exp + custom DVE 1-Newton
   reciprocal; no Identity/Square table churn (Identity shares the
   table).
 - Skewed schedule keeps the PE free of stalls so the HAM clock gate
   stays at K=8/8 (2.4 GHz): iter i runs conv(i), dw(i-2), stats(i-2),
   lse(i-3).  Cross-partition row copies for xcvL go over sbuf->sbuf
   DMA, off the engines.
"""
import numpy as np

import concourse.bass as bass
import concourse.bacc as bacc
import concourse.mybir as mybir
from concourse.tile import TileContext
from concourse.bass_utils import run_bass_kernel_spmd
from concourse.mybir import AluOpType, ActivationFunctionType

F32 = mybir.dt.float32
BF16 = mybir.dt.bfloat16

# ---- custom fused DVE ops ----
from concourse.dve_spec import (Spec, Src0, Src1, C0, C1, C2, Zero, One,
                                maxx, minn, sq, Bin, AluOp, lower)
from concourse.dve_spec import _has_src1 as _spec_has_src1
import concourse.dve_ops as _dve_ops
from concourse.dve_uop import DveOpSpec as _DveOpSpec


def _register_dve_op(name, spec):
    if name in _dve_ops._SUB_OPCODE_FOR_NAME:
        return next(op for op in _dve_ops.OPS if op.name == name)
    opcode = _dve_ops._CUSTOM_DVE_ROW_BASE + len(_dve_ops.OPS)
    shas = {}
    for ver in ("v3", "v4"):
        try:
            so = _DveOpSpec(name=name, opcode=opcode,
                            uops=lower(spec, ver=ver),
                            rd1_en=_spec_has_src1(spec))
            shas[ver] = so.sha(ver)
        except Exception:
            pass
    op = _dve_ops.DveOp(name, spec, subdim=False, uops_sha=shas)
    _dve_ops.OPS.append(op)
    _dve_ops._SUB_OPCODE_FOR_NAME[name] = opcode
    _dve_ops.CUSTOM_DVE_SPECS[name] = spec
    return op


# sq with per-partition bias: out = (in0 + s0)^2
OP_SQB = _register_dve_op(
    "ANT_SQB",
    Spec(body=sq(Src0 + C0),
         reference=lambda in0, in1, s0, s1, imm2: (in0 + s0) * (in0 + s0)))

# veps = (meansq + eps) - mean^2   [in0 = mean, in1 = meansq, imm2 = eps]
OP_VEPS = _register_dve_op(
    "ANT_VEPS",
    Spec(body=(Src1 + C2) - Src0 * Src0,
         reference=lambda in0, in1, s0, s1, imm2: (in1 + imm2) - in0 * in0))

# nrm = clamp((pd + cb)*isd + beta, -imm2, imm2)  [in0 = pd, in1 = isd']
OP_NRM2 = _register_dve_op(
    "ANT_NRM2",
    Spec(body=minn(maxx((Src0 + C0) * Src1 + C1, Zero - C2), C2),
         reference=lambda in0, in1, s0, s1, imm2:
             np.minimum(np.maximum((in0 + s0) * in1 + s1, -imm2), imm2)))

# y1 ~= 1/(1+in0): bitwise-not seed + one Newton step (s0, s1 tuned for
# minimax tanh error when used as tanh = 1 - 2*y1)
_X = Src0 + One
_NX = Bin(AluOp.BITWISE_NOT, _X, _X)
_Y0 = _NX * C0
OP_RCP1P = _register_dve_op(
    "ANT_RCP1P",
    Spec(body=_Y0 * (C1 - _X * _Y0),
         reference=lambda in0, in1, s0, s1, imm2: (lambda X: (lambda y0:
             y0 * (s1 - X * y0))((~X.astype(np.float32).view(np.int32))
             .view(np.float32) * s0))(in0 + 1.0)))

# home write: out = in0 + s0 (psum -> bf16)
OP_ADDB = _register_dve_op(
    "ANT_ADDB",
    Spec(body=Src0 + C0,
         reference=lambda in0, in1, s0, s1, imm2: in0 + s0))

# fused = (1 - 2r) * clip(nrm*s0 + s1, 0, 1)
OP_GATE = _register_dve_op(
    "ANT_TANH_GATE",
    Spec(body=((One - Src0) - Src0) * minn(maxx(Src1 * C0 + C1, Zero), One),
         reference=lambda in0, in1, s0, s1, imm2:
             (1.0 - 2.0 * in0) * np.clip(in1 * s0 + s1, 0.0, 1.0)))

AF = ActivationFunctionType

B, C, H, W = 16, 64, 256, 256
K = 3
G = 8
GS = C // G
EPS = 1e-05
NCORES = 8
BPC = B // NCORES          # samples per core

R = 4                      # output rows per block (2 per group)
WP = W + 4                 # padded input row width (2 left, 2 right)
WO = W + 2                 # xcvL row width (1 left, 1 right)
NBLK = H // R

_DIS = set()
TANH_C0 = -0.2362
TANH_C1 = 2.00165


_ACT_TABLE = "natural_log_exp_and_others"
_orig_get_tables = bacc.get_activation_tables


def _one_table(arch):
    tabs = _orig_get_tables(arch)
    return {name: (fns if name == _ACT_TABLE else set())
            for name, fns in tabs.items()}


bacc.get_activation_tables = _one_table


def _build_nc():
    import os as _os
    nc = bacc.Bacc("TRN2", target_bir_lowering=False)
    x = nc.dram_tensor("x", [BPC, 128, H + 4, WP], BF16, kind="ExternalInput")
    wc2 = nc.dram_tensor("wc2", [128, 4, 128], BF16, kind="ExternalInput")
    wc3 = nc.dram_tensor("wc3", [64, 4, 128], BF16, kind="ExternalInput")
    wdw = nc.dram_tensor("wdw", [128, 9, 128], BF16, kind="ExternalInput")
    wstat = nc.dram_tensor("wstat", [128, 128], BF16, kind="ExternalInput")
    wones = nc.dram_tensor("wones", [128, 2], BF16, kind="ExternalInput")
    cb2 = nc.dram_tensor("cb2", [128, 1], F32, kind="ExternalInput")
    bet2 = nc.dram_tensor("bet2", [128, 1], F32, kind="ExternalInput")
    lngam2 = nc.dram_tensor("lngam2", [128, 1], F32, kind="ExternalInput")
    out = nc.dram_tensor("out", [BPC, H * W], F32, kind="ExternalOutput")

    with TileContext(nc) as tc:
        with tc.tile_pool(name="consts", bufs=1) as consts, \
             tc.tile_pool(name="work", bufs=3) as work, \
             tc.tile_pool(name="outp", bufs=2) as outp, \
             tc.tile_pool(name="ppc", bufs=2, space="PSUM") as ppc, \
             tc.tile_pool(name="ppd", bufs=2, space="PSUM") as ppd, \
             tc.tile_pool(name="pmean", bufs=2, space="PSUM") as pmean_p, \
             tc.tile_pool(name="psq", bufs=1, space="PSUM") as psq_p, \
             tc.tile_pool(name="ppl", bufs=1, space="PSUM") as ppl:

            wc2t = consts.tile([128, 4, 128], BF16)
            wc3t = consts.tile([64, 4, 128], BF16)
            wdwt = consts.tile([128, 9, 128], BF16)
            wstatt = consts.tile([128, 128], BF16)
            wonest = consts.tile([128, 2], BF16)
            cb2t = consts.tile([128, 1], F32)
            bet2t = consts.tile([128, 1], F32)
            lngam2t = consts.tile([128, 1], F32)
            nc.sync.dma_start(out=wc2t, in_=wc2[:, :, :])
            nc.sync.dma_start(out=wc3t, in_=wc3[:, :, :])
            nc.sync.dma_start(out=wdwt, in_=wdw[:, :, :])
            nc.sync.dma_start(out=wstatt, in_=wstat[:, :])
            nc.sync.dma_start(out=wonest, in_=wones[:, :])
            nc.sync.dma_start(out=cb2t, in_=cb2[:, :])
            nc.sync.dma_start(out=bet2t, in_=bet2[:, :])
            nc.sync.dma_start(out=lngam2t, in_=lngam2[:, :])

            # manual ring tiles
            xin_r = [consts.tile([128, 6, WP], BF16, name=f"xin{r}")
                     for r in range(4)]
            xcv_r = [consts.tile([128, 5, WO], BF16, name=f"xcv{r}")
                     for r in range(4)]
            sq_r = [consts.tile([128, 2, W], BF16, name=f"sq{r}")
                    for r in range(4)]
            ez_r = [consts.tile([128, 2, W], BF16, name=f"ez{r}")
                    for r in range(4)]
            # zero the halo columns once; hot loop never writes them
            for t in xcv_r:
                nc.gpsimd.memset(t[:, :, 0:1], 0.0)
                nc.gpsimd.memset(t[:, :, WO - 1:WO], 0.0)

            _fl = lambda a: a.rearrange("p a b -> p (a b)")
            PS = lambda tile: tile.ap[0][0]    # partition stride

            def rows2(tile, row0, col0, parts=128):
                """AP [parts, 2, W]: rows row0, row0+2 of a ring tile."""
                rp = tile.ap[-2][0]
                return bass.AP(tensor=tile.tensor,
                               offset=tile.offset + row0 * rp + col0,
                               ap=[[PS(tile), parts], [2 * rp, 2], [1, W]])

            KREP = int(_os.environ.get("KREPEAT", "1"))

            NTOT = BPC * NBLK

            def _body():
                for i in range(NTOT + 4):
                    ii = i % 4
                    blk = i % NBLK          # block within sample
                    b = i // NBLK           # sample for conv stage
                    # ---- input prefetch (2 blocks ahead) ----
                    if i == 0:
                        for p in range(2):
                            nc.sync.dma_start(
                                out=xin_r[p],
                                in_=x[0, :, 4 * p + 1:4 * p + 7, :])
                    if i + 2 < NTOT:
                        pb, pk = divmod(i + 2, NBLK)
                        nc.sync.dma_start(
                            out=xin_r[(i + 2) % 4],
                            in_=x[pb, :, 4 * pk + 1:4 * pk + 7, :])

                    # ---- lse for block m = i-3 ----
                    m = i - 4
                    if 0 <= m < NTOT and "lse" not in _DIS:
                        bm, blkm = divmod(m, NBLK)
                        pl = ppl.tile([2, 2, W], F32, tag="pl")
                        nc.tensor.matmul(pl, wonest, ez_r[m % 4][:, :, :],
                                         start=True, stop=True)
                        lse = outp.tile([2, 2, W], F32, tag="lse")
                        nc.scalar.activation(lse, pl, AF.Ln)
                        nc.sync.dma_start(
                            out=out[bm, 4 * blkm * W:(4 * blkm + 4) * W
                                    ].rearrange("(a p c) -> p a c", p=2, c=W),
                            in_=lse)

                    # ---- conv(i): 8 passes -> pc ----
                    if i < NTOT:
                        xin = xin_r[ii]
                        pc = ppc.tile([128, 2, W], F32, tag="pc")
                        for t in range(4 if "conv" not in _DIS else 0):
                            rhs = bass.AP(
                                tensor=xin.tensor,
                                offset=xin.offset + t * WP + 2,
                                ap=[[PS(xin), 128], [2 * WP, 2], [1, W]])
                            nc.tensor.matmul(pc, wc2t[:, t, :], rhs,
                                             start=(t == 0), stop=False)
                        for t in range(4 if "conv" not in _DIS else 0):
                            rhs = bass.AP(
                                tensor=xin.tensor,
                                offset=xin.offset + t * WP + 3,
                                ap=[[PS(xin), 64], [2 * WP, 2], [1, W]])
                            nc.tensor.matmul(pc, wc3t[:, t, :], rhs,
                                             start=False, stop=(t == 3))

                        # home write: slots {1,3}, cols 1..W, + bias, bf16
                        xcv = xcv_r[ii]
                        if "home" not in _DIS:
                            nc.vector._custom_dve(OP_ADDB,
                                                  out=rows2(xcv, 1, 1),
                                                  in0=pc[:, :, :], s0=cb2t)
                        if "sqb" not in _DIS:
                            nc.scalar.activation(sq_r[ii], pc, AF.Square,
                                                 bias=cb2t)
                        # cross row copies (sbuf->sbuf DMA, partition shift)
                        if "crossdma" not in _DIS:
                            # g1 home rows {1,3} -> g0 slots {2,4}
                            nc.sync.dma_start(
                                out=rows2(xcv, 2, 1, 64),
                                in_=bass.AP(
                                    tensor=xcv.tensor,
                                    offset=xcv.offset + 64 * PS(xcv)
                                    + 1 * WO + 1,
                                    ap=[[PS(xcv), 64], [2 * WO, 2], [1, W]]))
                            # g0 home row {3} -> g1 slot {2}
                            nc.sync.dma_start(
                                out=bass.AP(tensor=xcv.tensor,
                                            offset=xcv.offset + 64 * PS(xcv)
                                            + 2 * WO + 1,
                                            ap=[[PS(xcv), 64], [1, W]]),
                                in_=bass.AP(tensor=xcv.tensor,
                                            offset=xcv.offset + 3 * WO + 1,
                                            ap=[[PS(xcv), 64], [1, W]]))
                            # this home row {1} -> PREV tile g1 slot {4}
                            if i >= 1 and blk >= 1:
                                pxcv = xcv_r[(i - 1) % 4]
                                nc.sync.dma_start(
                                    out=bass.AP(tensor=pxcv.tensor,
                                                offset=pxcv.offset
                                                + 64 * PS(pxcv) + 4 * WO + 1,
                                                ap=[[PS(pxcv), 64], [1, W]]),
                                    in_=bass.AP(tensor=xcv.tensor,
                                                offset=xcv.offset + 1 * WO + 1,
                                                ap=[[PS(xcv), 64], [1, W]]))
                        # sample seam / image top: no previous row
                        if "prevcopy" not in _DIS:
                            if blk == 0:
                                nc.gpsimd.memset(xcv[:, 0:1, :], 0.0)
                            else:
                                nc.sync.dma_start(
                                    out=xcv[:, 0:1, :],
                                    in_=xcv_r[(i - 1) % 4][:, 4:5, :])
                    if (i >= NTOT or blk == 0) and i >= 1:
                        # previous tile was last block of a sample:
                        # its g1 slot 4 = image row 256 -> zero
                        pxcv = xcv_r[(i - 1) % 4]
                        nc.gpsimd.memset(pxcv[64:128, 4:5, 1:WO - 1], 0.0)

                    # ---- chain for block k = i-2 ----
                    k = i - 2
                    if 0 <= k < NTOT and "tail" not in _DIS:
                        kk = k % 4
                        xcvk = xcv_r[kk]
                        pd = ppd.tile([128, 2, W], F32, tag="pd")
                        for t9 in range(9 if "dw" not in _DIS else 0):
                            dy, dx = divmod(t9, 3)
                            rhs = bass.AP(
                                tensor=xcvk.tensor,
                                offset=xcvk.offset + dy * WO + dx,
                                ap=[[PS(xcvk), 128], [2 * WO, 2], [1, W]])
                            nc.tensor.matmul(pd, wdwt[:, t9, :], rhs,
                                             start=(t9 == 0), stop=(t9 == 8))
                        pmean = pmean_p.tile([128, 2, W], F32, tag="pm")
                        psq = psq_p.tile([128, 2, W], F32, tag="pq")
                        if "stats" not in _DIS:
                            nc.tensor.matmul(pmean, wstatt, rows2(xcvk, 1, 1),
                                             start=True, stop=True)
                            nc.tensor.matmul(psq, wstatt, sq_r[kk][:, :, :],
                                             start=True, stop=True)

                        m2 = work.tile([128, 2, W], F32, tag="m2")
                        nc.scalar.activation(m2, pmean, AF.Square)
                        veps = work.tile([128, 2, W], F32, tag="veps")
                        nc.vector.scalar_tensor_tensor(
                            out=veps, in0=psq, scalar=EPS, in1=m2,
                            op0=AluOpType.add, op1=AluOpType.subtract)
                        lnv = work.tile([128, 2, W], F32, tag="lnv")
                        nc.scalar.activation(lnv, veps, AF.Ln)
                        isd = work.tile([128, 2, W], F32, tag="isd")
                        nc.scalar.activation(isd, lnv, AF.Exp,
                                             bias=lngam2t, scale=-0.5)
                        nrm = work.tile([128, 2, W], F32, tag="nrm")
                        nc.vector._custom_dve(OP_NRM2, out=_fl(nrm),
                                              in0=_fl(pd), in1=_fl(isd),
                                              s0=cb2t, s1=bet2t, imm2=30.0)
                        ee = work.tile([128, 2, W], F32, tag="ee")
                        nc.scalar.activation(ee, nrm, AF.Exp, scale=2.0)
                        rr = work.tile([128, 2, W], F32, tag="rr")
                        nc.vector._custom_dve(OP_RCP1P, out=_fl(rr),
                                              in0=_fl(ee),
                                              s0=TANH_C0, s1=TANH_C1)
                        zz = work.tile([128, 2, W], F32, tag="zz")
                        nc.vector._custom_dve(OP_GATE, out=_fl(zz),
                                              in0=_fl(rr), in1=_fl(nrm),
                                              s0=1.0 / 6.0, s1=0.5)
                        nc.gpsimd.tensor_tensor(zz, zz, rows2(xcvk, 1, 1),
                                                op=AluOpType.add)
                        nc.scalar.activation(ez_r[kk], zz, AF.Exp)
            for _rep in range(KREP):
                _body()
    nc.compile()
    return nc


def _host_weights(conv_w, conv_b, gn_scale, gn_bias):
    w = np.asarray(conv_w, np.float32)          # [co, ci, dy, dx]
    gsel = np.zeros((C, C), np.float32)
    for g in range(G):
        gsel[g * GS:(g + 1) * GS, g * GS:(g + 1) * GS] = 1.0 / GS

    # conv: wc2[t][(ci,h), (co,g)] = w[co, ci, t-g, 1-h]
    wc2 = np.zeros((128, 4, 128), np.float32)
    wc3 = np.zeros((64, 4, 128), np.float32)
    for t in range(4):
        for g in range(2):
            dy = t - g
            if 0 <= dy <= 2:
                wc2[0:64, t, g * 64:(g + 1) * 64] = w[:, :, dy, 1].T
                wc2[64:128, t, g * 64:(g + 1) * 64] = w[:, :, dy, 0].T
                wc3[:, t, g * 64:(g + 1) * 64] = w[:, :, dy, 2].T

    wdiag = np.einsum('cckl->ckl', w)           # [C, 3, 3]
    wdw = np.zeros((128, 9, 128), np.float32)
    for dy in range(3):
        for dx in range(3):
            mmat = np.diag(wdiag[:, dy, dx]).astype(np.float32)
            if dy == 1 and dx == 1:
                mmat = mmat - gsel
            t9 = dy * 3 + dx
            wdw[0:64, t9, 0:64] = mmat
            wdw[64:128, t9, 64:128] = mmat

    wstat = np.zeros((128, 128), np.float32)
    wstat[0:64, 0:64] = gsel
    wstat[64:128, 64:128] = gsel

    wones = np.zeros((128, 2), np.float32)
    wones[0:64, 0] = 1.0
    wones[64:128, 1] = 1.0

    cb = np.asarray(conv_b, np.float32)
    gam = np.asarray(gn_scale, np.float32)
    bet = np.asarray(gn_bias, np.float32)
    cb2 = np.concatenate([cb, cb]).reshape(128, 1)
    bet2 = np.concatenate([bet, bet]).reshape(128, 1)
    lngam2 = np.log(np.concatenate([gam, gam])).reshape(128, 1).astype(np.float32)

    import ml_dtypes
    bf = ml_dtypes.bfloat16
    return dict(wc2=wc2.astype(bf), wc3=wc3.astype(bf), wdw=wdw.astype(bf),
                wstat=wstat.astype(bf), wones=wones.astype(bf),
                cb2=cb2, bet2=bet2, lngam2=lngam2)


_NC_CACHE = None


def kernel(x, conv_w, conv_b, gn_scale, gn_bias):
    global _NC_CACHE
    x = np.asarray(x, np.float32)
    wts = _host_weights(conv_w, conv_b, gn_scale, gn_bias)
    if _NC_CACHE is None:
        _NC_CACHE = _build_nc()
    nc = _NC_CACHE
    import ml_dtypes
    xpad = np.zeros((B, 128, H + 4, WP), ml_dtypes.bfloat16)
    xpad[:, 0:64, 2:2 + H, 2:2 + W] = x
    xpad[:, 64:128, 2:2 + H, 3:3 + W] = x
    in_maps = []
    for c in range(NCORES):
        m = {"x": np.ascontiguousarray(xpad[c * BPC:(c + 1) * BPC])}
        m.update(wts)
        in_maps.append(m)
    import os as _os
    trace = bool(int(_os.environ.get("KTRACE", "0")))
    res = run_bass_kernel_spmd(nc, in_maps, core_ids=list(range(NCORES)),
                               trace=trace)
    kernel.exec_time_ns = res.exec_time_ns
    kernel.results_obj = res
    outs = [res.results[c]["out"].reshape(BPC, 1, H, W) for c in range(NCORES)]
    return np.concatenate(outs, axis=0)


if __name__ == "__main__":
    rng = np.random.default_rng(0)
    xs = rng.standard_normal((B, C, H, W), dtype=np.float32)
    wv = (rng.standard_normal((C, C, K, K), dtype=np.float32)
          / np.sqrt(C * K * K)).astype(np.float32)
    bv = (rng.standard_normal(C) * 0.05).astype(np.float32)
    gv = (1 + 0.05 * rng.standard_normal(C)).astype(np.float32)
    btv = (0.05 * rng.standard_normal(C)).astype(np.float32)
    o = kernel(xs, wv, bv, gv, btv)
    print(o.shape, o.dtype, float(o.mean()))
